# revision 1
# baseline (speedup 1.0000x reference)
"""Multi-head attention (B=4, S=2048, H=16, D=64) on 8 TRN2 NeuronCores.

64 independent (b, h) attention slices, 8 per core.  Per slice, per
512-wide query block (iblk):

  mm1 (PE):   S^T[k, i] = K^T_tile.T @ Q^T_block   (contract d=64,
              alternating partition halves -> alternating PE row groups)
  exp:        E^T = exp(0.125 * S^T), PSUM -> SBUF bf16, split across
              engines by k-group:
                ACT groups: exact exp activation
                DVE groups: phase-averaged Schraudolph in bf16-bit space
                  y1 = int16(a * s + b)            (DVE, PSUM read)
                  y2 = y1 + 64                     (GPSIMD, SBUF only)
                  E^T = bf16(y1) + bf16(y2)        (DVE)
  mm2 (PE):   flipped, E-stationary: for each 128-query sub-block,
              acc[i, 0:65] += E^T_tile[:, i-sub].T @ Vext_tile
              (Vext = [V | ones], col 64 = softmax row-sum), 4 sequential
              accumulation groups packed in one PSUM bank.
  out:        DMA raw acc (unnormalized + row sums) to DRAM; the host
              divides by the row sums and reassembles [B,S,H,D].
"""

import numpy as np

import concourse.bass as bass  # noqa: F401
import concourse.mybir as mybir
import concourse.tile as tile
from concourse import bacc
from concourse.bass_utils import run_bass_kernel_spmd

B, S, H, D = 4, 2048, 16, 64
N_CORES = 8
SLICES = B * H              # 64 independent attention slices
SPC = SLICES // N_CORES     # 8 slices per core
KT = S // 128               # 16 key tiles of 128
IBLK = 512                  # query block width
NIB = S // IBLK             # 4 query blocks per slice
NG = 8                      # k-groups of 2 tiles per iblk
F32 = mybir.dt.float32
BF16 = mybir.dt.bfloat16
I16 = mybir.dt.int16

# Schraudolph constants (bf16 bit space), calibrated for the
# trunc(f32->int16) convert.  'P' = phase-pair (avg of two half-ulp-
# shifted approximants, ~0.6% rms), 'S' = single (1.8% rms).
A_DEV = 23.083120654223414   # 128 * log2(e) / 8
B_P2 = 16086.0               # 16256 - 128 + C*, C* = -42
B_S1 = 16249.0               # 16256 + C*, C* = -7

# Per-iblk engine assignment for the 8 exp groups: 'A' = ACT exact exp,
# 'S' = 1-op Schraudolph (DVE), 'P' = phase-pair (DVE ts + GPSIMD
# y2/add).  Cycle averages 4.75 A / 1.75 S / 1.5 P; approx groups sit
# early (long cross-engine chains get a head start) but not first
# (mm2 consumes group 0's output first).
# Every group is split: ACT exact exp on cols 0:512 (PSUM bank A) in
# parallel with DVE approx on cols 512:1024 (bank B).  'X' = phase-pair
# DVE half (y2/add on GPSIMD), 'Z' = 1-op Schraudolph DVE half.
PATTERNS = [
    "XZXZXZXZ",
]

_CACHE = {}


def _build(patterns=None):
    patterns = patterns or PATTERNS
    nc = bacc.Bacc("TRN2", target_bir_lowering=False, debug=False)

    qt_d = nc.declare_dram_parameter("qt", [SPC, 128, S], BF16, isOutput=False).ap()
    kt_d = nc.declare_dram_parameter("kt", [SPC, 128, S], BF16, isOutput=False).ap()
    vx_d = nc.declare_dram_parameter("vx", [SPC, 128, KT, 65], BF16, isOutput=False).ap()
    out_d = nc.declare_dram_parameter("out", [SPC, NIB, 128, 4 * 65], F32, isOutput=True).ap()

    EXP = mybir.ActivationFunctionType.Exp
    ADD = mybir.AluOpType.add
    MUL = mybir.AluOpType.mult

    with tile.TileContext(nc) as tc:
        with (
            tc.tile_pool(name="qk", bufs=SPC) as qk_pool,
            tc.tile_pool(name="vp", bufs=SPC) as v_pool,
            tc.tile_pool(name="et", bufs=26) as e_pool,
            tc.tile_pool(name="yp", bufs=10) as y_pool,
            tc.tile_pool(name="ob", bufs=3) as o_pool,
            tc.tile_pool(name="stg", bufs=6, space="PSUM") as stg_pool,
            tc.tile_pool(name="acc", bufs=2, space="PSUM") as acc_pool,
        ):
            pending = []
            ibc = 0  # global iblk counter for pattern cycling

            copy_jobs = []

            def emit_copy(cjob):
                s, ib, acc = cjob
                o_sb = o_pool.tile([128, 4 * 65], F32, tag="osb")
                nc.scalar.copy(o_sb[:], acc[:])
                nc.sync.dma_start(out_d[s, ib], o_sb[:])

            def mm2_chunks(job):
                """Yield the 64 mm2 matmuls of one iblk as 8 emission
                chunks of 8, so they interleave with the next iblk's mm1
                stages and keep PE paced with the exp consumers."""
                s, ib, ets, vx_sb, pat = job
                acc = acc_pool.tile([128, 4 * 65], F32, tag="acc")
                # consume ACT-produced ets first; the multi-hop approx
                # chains (DVE -> Pool -> Pool) finish later
                # even k = ACT half (1 hop), odd k = DVE/Pool half; the
                # phase-pair ('X') odd halves land last
                def _klat(k):
                    if k % 2 == 0:
                        return 0
                    return 2 if pat[k // 2] == "X" else 1

                korder = sorted(range(KT), key=_klat)
                items = [
                    (isub, n, k)
                    for isub in range(4)
                    for n, k in enumerate(korder)
                ]
                emitters = []
                for c in range(8):
                    def emit(chunk=items[c * 8 : (c + 1) * 8]):
                        for isub, n, k in chunk:
                            g, j = divmod(k, 2)
                            q0 = j * IBLK + isub * 128
                            nc.tensor.matmul(
                                acc[:, isub * 65 : isub * 65 + 65],
                                lhsT=ets[g][:, q0 : q0 + 128],
                                rhs=vx_sb[:, k, :],
                                start=(n == 0),
                                stop=(n == KT - 1),
                            )
                    emitters.append(emit)
                copy_jobs.append((s, ib, acc))
                return emitters

            # Prefetch every slice's inputs up front (fits in SBUF) so
            # slice boundaries never stall on DMA.
            qt_sbs, kt_sbs, vx_sbs = [], [], []
            for s in range(SPC):
                qt_sb = qk_pool.tile([128, S], BF16, tag="qt")
                kt_sb = qk_pool.tile([128, S], BF16, tag="kt")
                vx_sb = v_pool.tile([128, KT, 65], BF16, tag="vx")
                if s == 0:
                    # chunked first-slice loads so the first mm1 groups
                    # start as soon as their kt/qt columns land
                    for c in range(4):
                        nc.sync.dma_start(
                            kt_sb[:, c * 512 : (c + 1) * 512],
                            kt_d[s, :, c * 512 : (c + 1) * 512],
                        )
                        nc.sync.dma_start(
                            qt_sb[:, c * 512 : (c + 1) * 512],
                            qt_d[s, :, c * 512 : (c + 1) * 512],
                        )
                else:
                    nc.sync.dma_start(qt_sb[:], qt_d[s])
                    nc.sync.dma_start(kt_sb[:], kt_d[s])
                nc.sync.dma_start(vx_sb[:], vx_d[s])
                qt_sbs.append(qt_sb)
                kt_sbs.append(kt_sb)
                vx_sbs.append(vx_sb)

            mm2_list = []  # chunk emitters for the in-flight mm2 iblk

            for s in range(SPC):
                qt_sb, kt_sb, vx_sb = qt_sbs[s], kt_sbs[s], vx_sbs[s]
                for ib in range(NIB):
                    i0 = ib * IBLK
                    pat = patterns[ibc % len(patterns)]
                    ibc += 1
                    ets = []
                    for g in range(NG):
                        if mm2_list:
                            mm2_list.pop(0)()
                        stga = stg_pool.tile([128, IBLK], F32, tag="stg")
                        stgb = stg_pool.tile([128, IBLK], F32, tag="stg")
                        for j, stg in enumerate((stga, stgb)):
                            k = 2 * g + j
                            p0 = (k % 2) * D
                            nc.tensor.matmul(
                                stg[:],
                                lhsT=kt_sb[p0 : p0 + D, k * 128 : (k + 1) * 128],
                                rhs=qt_sb[p0 : p0 + D, i0 : i0 + IBLK],
                                start=True,
                                stop=True,
                            )
                        et = e_pool.tile([128, 2 * IBLK], BF16, tag="et")
                        nc.scalar.activation(
                            et[:, :IBLK], stga[:], EXP, scale=0.125
                        )
                        if pat[g] == "Z":
                            nc.vector.tensor_scalar(
                                et[:, IBLK:].bitcast(I16),
                                stgb[:],
                                A_DEV,
                                B_S1,
                                op0=MUL,
                                op1=ADD,
                            )
                        else:
                            y1 = y_pool.tile([128, IBLK], I16, tag="y1")
                            nc.vector.tensor_scalar(
                                y1[:], stgb[:], A_DEV, B_P2, op0=MUL, op1=ADD
                            )
                            y2 = y_pool.tile([128, IBLK], I16, tag="y2")
                            nc.gpsimd.tensor_scalar(y2[:], y1[:], 64, None, op0=ADD)
                            nc.gpsimd.tensor_tensor(
                                et[:, IBLK:], y1[:].bitcast(BF16), y2[:].bitcast(BF16), ADD
                            )
                        ets.append(et)
                    pending.append((s, ib, ets, vx_sb, pat))
                    if len(pending) > 1:
                        mm2_list = mm2_chunks(pending.pop(0))
                    if len(copy_jobs) > 1:
                        emit_copy(copy_jobs.pop(0))

            while mm2_list:
                mm2_list.pop(0)()
            while pending:
                for e in mm2_chunks(pending.pop(0)):
                    e()
            while copy_jobs:
                emit_copy(copy_jobs.pop(0))

    nc.compile()
    return nc


import ml_dtypes  # noqa: E402

BF16_NP = ml_dtypes.bfloat16


def _prep(qw, kw, vw):
    """Host-side layout prep: per-slice transposed views, contiguous."""
    qw = np.asarray(qw, dtype=np.float32)
    kw = np.asarray(kw, dtype=np.float32)
    vw = np.asarray(vw, dtype=np.float32)

    def to_t(x):  # [B, S, H*D] -> [SLICES, D, S]
        x4 = x.reshape(B, S, H, D)
        return np.ascontiguousarray(
            x4.transpose(0, 2, 3, 1).reshape(SLICES, D, S)
        )

    qt = to_t(qw).astype(BF16_NP)
    kt = to_t(kw).astype(BF16_NP)
    qt = np.ascontiguousarray(np.concatenate([qt, qt], axis=1))  # [SLICES,128,S]
    kt = np.ascontiguousarray(np.concatenate([kt, kt], axis=1))
    v4 = vw.reshape(B, S, H, D).transpose(0, 2, 1, 3)  # [B, H, S, D]
    v5 = v4.reshape(SLICES, KT, 128, D)
    vx = np.empty((SLICES, KT, 128, 65), BF16_NP)
    vx[..., :D] = v5.astype(BF16_NP)
    vx[..., D] = 1.0
    vx = np.ascontiguousarray(vx.transpose(0, 2, 1, 3))  # [SLICES, 128, KT, 65]
    return qt, kt, vx


def _core_in_map(pre, c):
    qt, kt, vx = pre
    return {
        "qt": qt[c * SPC : (c + 1) * SPC],
        "kt": kt[c * SPC : (c + 1) * SPC],
        "vx": vx[c * SPC : (c + 1) * SPC],
    }


def _postprocess(outs):
    """[N_CORES, SPC, NIB, 128, 260] raw acc -> [B, S, H, D] normalized."""
    o = np.asarray(outs, dtype=np.float32).reshape(SLICES, NIB, 128, 4, 65)
    o = o.transpose(0, 1, 3, 2, 4).reshape(SLICES, S, 65)
    res = o[..., :D] / o[..., D:65]
    res = res.reshape(B, H, S, D).transpose(0, 2, 1, 3)
    return np.ascontiguousarray(res)


def kernel(qw, kw, vw):
    if "nc" not in _CACHE:
        _CACHE["nc"] = _build()
    nc = _CACHE["nc"]

    pre = _prep(qw, kw, vw)
    in_maps = [_core_in_map(pre, c) for c in range(N_CORES)]
    res = run_bass_kernel_spmd(nc, in_maps, core_ids=list(range(N_CORES)))
    outs = np.stack([np.asarray(res.results[c]["out"]) for c in range(N_CORES)])
    return _postprocess(outs)



# revision 4
# speedup vs baseline: 1.0010x; 1.0010x over previous
"""Multi-head attention (B=4, S=2048, H=16, D=64) on 8 TRN2 NeuronCores.

64 independent (b, h) attention slices, 8 per core.  Per slice, per
512-wide query block (iblk):

  mm1 (PE):   S^T[k, i] = K^T_tile.T @ Q^T_block   (contract d=64,
              alternating partition halves -> alternating PE row groups)
  exp:        E^T = exp(0.125 * S^T), PSUM -> SBUF bf16, split across
              engines by k-group:
                ACT groups: exact exp activation
                DVE groups: phase-averaged Schraudolph in bf16-bit space
                  y1 = int16(a * s + b)            (DVE, PSUM read)
                  y2 = y1 + 64                     (GPSIMD, SBUF only)
                  E^T = bf16(y1) + bf16(y2)        (DVE)
  mm2 (PE):   flipped, E-stationary: for each 128-query sub-block,
              acc[i, 0:65] += E^T_tile[:, i-sub].T @ Vext_tile
              (Vext = [V | ones], col 64 = softmax row-sum), 4 sequential
              accumulation groups packed in one PSUM bank.
  out:        DMA raw acc (unnormalized + row sums) to DRAM; the host
              divides by the row sums and reassembles [B,S,H,D].
"""

import numpy as np

import concourse.bass as bass  # noqa: F401
import concourse.mybir as mybir
import concourse.tile as tile
from concourse import bacc
from concourse.bass_utils import run_bass_kernel_spmd

B, S, H, D = 4, 2048, 16, 64
N_CORES = 8
SLICES = B * H              # 64 independent attention slices
SPC = SLICES // N_CORES     # 8 slices per core
KT = S // 128               # 16 key tiles of 128
IBLK = 512                  # query block width
NIB = S // IBLK             # 4 query blocks per slice
NG = 8                      # k-groups of 2 tiles per iblk
F32 = mybir.dt.float32
BF16 = mybir.dt.bfloat16
I16 = mybir.dt.int16

# Schraudolph constants (bf16 bit space), calibrated for the
# trunc(f32->int16) convert.  'P' = phase-pair (avg of two half-ulp-
# shifted approximants, ~0.6% rms), 'S' = single (1.8% rms).
A_DEV = 23.083120654223414   # 128 * log2(e) / 8
B_P2 = 16086.0               # 16256 - 128 + C*, C* = -42
B_S1 = 16249.0               # 16256 + C*, C* = -7

# Per-iblk engine assignment for the 8 exp groups: 'A' = ACT exact exp,
# 'S' = 1-op Schraudolph (DVE), 'P' = phase-pair (DVE ts + GPSIMD
# y2/add).  Cycle averages 4.75 A / 1.75 S / 1.5 P; approx groups sit
# early (long cross-engine chains get a head start) but not first
# (mm2 consumes group 0's output first).
# Every group is split: ACT exact exp on cols 0:512 (PSUM bank A) in
# parallel with DVE approx on cols 512:1024 (bank B).  'X' = phase-pair
# DVE half (y2/add on GPSIMD), 'Z' = 1-op Schraudolph DVE half.
PATTERNS = [
    "XZXZXZXZ",
]

_CACHE = {}


def _build(patterns=None):
    patterns = patterns or PATTERNS
    nc = bacc.Bacc("TRN2", target_bir_lowering=False, debug=False)

    # q/k are two-level fp8 packs [SPC, 128, 2, S] (int8 payload; the
    # axon param bridge lacks fp8): partition p<64 carries (hi, lo) of
    # d=p for q and hi of d=p for k; p>=64 duplicates q and carries k's
    # lo.  One DoubleRow matmul then computes the EXACT product
    # (k_hi+k_lo)^T (q_hi+q_lo) -- more accurate than bf16 at 0.5
    # cycles/row (half the bf16 mm1 PE cost).
    I8 = mybir.dt.int8
    F8E4 = mybir.dt.float8e4
    DRM = mybir.MatmulPerfMode.DoubleRow
    qt_d = nc.declare_dram_parameter("qt", [SPC, 128, 2, S], I8, isOutput=False).ap()
    kt_d = nc.declare_dram_parameter("kt", [SPC, 128, 2, S], I8, isOutput=False).ap()
    vx_d = nc.declare_dram_parameter("vx", [SPC, 128, KT, 65], BF16, isOutput=False).ap()
    out_d = nc.declare_dram_parameter("out", [SPC, NIB, 128, 4 * 65], F32, isOutput=True).ap()

    EXP = mybir.ActivationFunctionType.Exp
    ADD = mybir.AluOpType.add
    MUL = mybir.AluOpType.mult

    with tile.TileContext(nc) as tc:
        with (
            tc.tile_pool(name="qk", bufs=SPC) as qk_pool,
            tc.tile_pool(name="vp", bufs=SPC) as v_pool,
            tc.tile_pool(name="et", bufs=26) as e_pool,
            tc.tile_pool(name="yp", bufs=10) as y_pool,
            tc.tile_pool(name="ob", bufs=3) as o_pool,
            tc.tile_pool(name="stg", bufs=6, space="PSUM") as stg_pool,
            tc.tile_pool(name="acc", bufs=2, space="PSUM") as acc_pool,
        ):
            pending = []
            ibc = 0  # global iblk counter for pattern cycling

            copy_jobs = []

            def emit_copy(cjob):
                s, ib, acc = cjob
                o_sb = o_pool.tile([128, 4 * 65], F32, tag="osb")
                nc.scalar.copy(o_sb[:], acc[:])
                nc.sync.dma_start(out_d[s, ib], o_sb[:])

            def mm2_chunks(job):
                """Yield the 64 mm2 matmuls of one iblk as 8 emission
                chunks of 8, so they interleave with the next iblk's mm1
                stages and keep PE paced with the exp consumers."""
                s, ib, ets, vx_sb, pat = job
                acc = acc_pool.tile([128, 4 * 65], F32, tag="acc")
                # consume ACT-produced ets first; the multi-hop approx
                # chains (DVE -> Pool -> Pool) finish later
                # even k = ACT half (1 hop), odd k = DVE/Pool half; the
                # phase-pair ('X') odd halves land last
                def _klat(k):
                    if k % 2 == 0:
                        return 0
                    return 2 if pat[k // 2] == "X" else 1

                korder = sorted(range(KT), key=_klat)
                items = [
                    (isub, n, k)
                    for isub in range(4)
                    for n, k in enumerate(korder)
                ]
                emitters = []
                for c in range(8):
                    def emit(chunk=items[c * 8 : (c + 1) * 8]):
                        for isub, n, k in chunk:
                            g, j = divmod(k, 2)
                            q0 = j * IBLK + isub * 128
                            nc.tensor.matmul(
                                acc[:, isub * 65 : isub * 65 + 65],
                                lhsT=ets[g][:, q0 : q0 + 128],
                                rhs=vx_sb[:, k, :],
                                start=(n == 0),
                                stop=(n == KT - 1),
                            )
                    emitters.append(emit)
                copy_jobs.append((s, ib, acc))
                return emitters

            # Prefetch every slice's inputs up front (fits in SBUF) so
            # slice boundaries never stall on DMA.
            qt_sbs, kt_sbs, vx_sbs = [], [], []
            for s in range(SPC):
                qt_sb = qk_pool.tile([128, 2, S], I8, tag="qt")
                kt_sb = qk_pool.tile([128, 2, S], I8, tag="kt")
                vx_sb = v_pool.tile([128, KT, 65], BF16, tag="vx")
                if s == 0:
                    # chunked first-slice loads so the first mm1 groups
                    # start as soon as their kt/qt columns land
                    for c in range(4):
                        nc.sync.dma_start(
                            kt_sb[:, :, c * 512 : (c + 1) * 512],
                            kt_d[s, :, :, c * 512 : (c + 1) * 512],
                        )
                        nc.sync.dma_start(
                            qt_sb[:, :, c * 512 : (c + 1) * 512],
                            qt_d[s, :, :, c * 512 : (c + 1) * 512],
                        )
                else:
                    nc.sync.dma_start(qt_sb[:], qt_d[s])
                    nc.sync.dma_start(kt_sb[:], kt_d[s])
                nc.sync.dma_start(vx_sb[:], vx_d[s])
                qt_sbs.append(qt_sb)
                kt_sbs.append(kt_sb)
                vx_sbs.append(vx_sb)

            mm2_list = []  # chunk emitters for the in-flight mm2 iblk

            for s in range(SPC):
                qt_sb, kt_sb, vx_sb = qt_sbs[s], kt_sbs[s], vx_sbs[s]
                for ib in range(NIB):
                    i0 = ib * IBLK
                    pat = patterns[ibc % len(patterns)]
                    ibc += 1
                    ets = []
                    for g in range(NG):
                        if mm2_list:
                            mm2_list.pop(0)()
                        stga = stg_pool.tile([128, IBLK], F32, tag="stg")
                        stgb = stg_pool.tile([128, IBLK], F32, tag="stg")
                        for j, stg in enumerate((stga, stgb)):
                            k = 2 * g + j
                            nc.tensor.matmul(
                                stg[:],
                                lhsT=kt_sb[:, :, k * 128 : (k + 1) * 128].bitcast(F8E4),
                                rhs=qt_sb[:, :, i0 : i0 + IBLK].bitcast(F8E4),
                                start=True,
                                stop=True,
                                perf_mode=DRM,
                            )
                        et = e_pool.tile([128, 2 * IBLK], BF16, tag="et")
                        nc.scalar.activation(
                            et[:, :IBLK], stga[:], EXP, scale=0.125
                        )
                        if pat[g] == "Z":
                            nc.vector.tensor_scalar(
                                et[:, IBLK:].bitcast(I16),
                                stgb[:],
                                A_DEV,
                                B_S1,
                                op0=MUL,
                                op1=ADD,
                            )
                        else:
                            y1 = y_pool.tile([128, IBLK], I16, tag="y1")
                            nc.vector.tensor_scalar(
                                y1[:], stgb[:], A_DEV, B_P2, op0=MUL, op1=ADD
                            )
                            y2 = y_pool.tile([128, IBLK], I16, tag="y2")
                            nc.gpsimd.tensor_scalar(y2[:], y1[:], 64, None, op0=ADD)
                            nc.gpsimd.tensor_tensor(
                                et[:, IBLK:], y1[:].bitcast(BF16), y2[:].bitcast(BF16), ADD
                            )
                        ets.append(et)
                    pending.append((s, ib, ets, vx_sb, pat))
                    if len(pending) > 1:
                        mm2_list = mm2_chunks(pending.pop(0))
                    if len(copy_jobs) > 1:
                        emit_copy(copy_jobs.pop(0))

            while mm2_list:
                mm2_list.pop(0)()
            while pending:
                for e in mm2_chunks(pending.pop(0)):
                    e()
            while copy_jobs:
                emit_copy(copy_jobs.pop(0))

    nc.compile()
    return nc


import ml_dtypes  # noqa: E402

BF16_NP = ml_dtypes.bfloat16
F8_NP = ml_dtypes.float8_e4m3


def _prep(qw, kw, vw):
    """Host-side layout prep: per-slice transposed views, contiguous."""
    qw = np.asarray(qw, dtype=np.float32)
    kw = np.asarray(kw, dtype=np.float32)
    vw = np.asarray(vw, dtype=np.float32)

    def to_t(x):  # [B, S, H*D] -> [SLICES, D, S]
        x4 = x.reshape(B, S, H, D)
        return np.ascontiguousarray(
            x4.transpose(0, 2, 3, 1).reshape(SLICES, D, S)
        )

    def two_level_pack(x, dup_hi_lo):
        """x [SLICES, 64, S] f32 -> [SLICES, 128, 2, S] fp8 (int8 view).

        dup_hi_lo=False (q): rows 0:64 = (hi, lo) on the pair dim,
            rows 64:128 duplicate rows 0:64.
        dup_hi_lo=True (k): rows 0:64 = hi (both pair slots),
            rows 64:128 = lo (both pair slots).
        """
        hi = x.astype(F8_NP)
        lo = (x - hi.astype(np.float32)).astype(F8_NP)
        out = np.empty((SLICES, 128, 2, x.shape[2]), F8_NP)
        if dup_hi_lo:
            out[:, :64, 0] = hi
            out[:, :64, 1] = hi
            out[:, 64:, 0] = lo
            out[:, 64:, 1] = lo
        else:
            out[:, :64, 0] = hi
            out[:, :64, 1] = lo
            out[:, 64:, 0] = hi
            out[:, 64:, 1] = lo
        return np.ascontiguousarray(out).view(np.int8)

    qt = two_level_pack(to_t(qw), dup_hi_lo=False)  # [SLICES,128,2,S] i8
    kt = two_level_pack(to_t(kw), dup_hi_lo=True)
    v4 = vw.reshape(B, S, H, D).transpose(0, 2, 1, 3)  # [B, H, S, D]
    v5 = v4.reshape(SLICES, KT, 128, D)
    vx = np.empty((SLICES, KT, 128, 65), BF16_NP)
    vx[..., :D] = v5.astype(BF16_NP)
    vx[..., D] = 1.0
    vx = np.ascontiguousarray(vx.transpose(0, 2, 1, 3))  # [SLICES, 128, KT, 65]
    return qt, kt, vx


def _core_in_map(pre, c):
    qt, kt, vx = pre
    return {
        "qt": qt[c * SPC : (c + 1) * SPC],
        "kt": kt[c * SPC : (c + 1) * SPC],
        "vx": vx[c * SPC : (c + 1) * SPC],
    }


def _postprocess(outs):
    """[N_CORES, SPC, NIB, 128, 260] raw acc -> [B, S, H, D] normalized."""
    o = np.asarray(outs, dtype=np.float32).reshape(SLICES, NIB, 128, 4, 65)
    o = o.transpose(0, 1, 3, 2, 4).reshape(SLICES, S, 65)
    res = o[..., :D] / o[..., D:65]
    res = res.reshape(B, H, S, D).transpose(0, 2, 1, 3)
    return np.ascontiguousarray(res)


def kernel(qw, kw, vw):
    if "nc" not in _CACHE:
        _CACHE["nc"] = _build()
    nc = _CACHE["nc"]

    pre = _prep(qw, kw, vw)
    in_maps = [_core_in_map(pre, c) for c in range(N_CORES)]
    res = run_bass_kernel_spmd(nc, in_maps, core_ids=list(range(N_CORES)))
    outs = np.stack([np.asarray(res.results[c]["out"]) for c in range(N_CORES)])
    return _postprocess(outs)

